# revision 49
# baseline (speedup 1.0000x reference)
"""NonLocalBlock2D (embedded-gaussian non-local attention) on 8 TRN2 NeuronCores.

Sharding: data-parallel over (batch, query-half). Core k handles sample b=k//2,
query rows h*3200:(h+1)*3200 with h=k%2. Attention keys/values are the full
6400 positions of that sample; params are replicated (folded host-side).

Key structure (v2, tuned against the InstructionCostModel):
  - theta/phi convs fused:  f = xa^T (Theta^T Phi) xa  with xa = [x; 1]
    (bias rows folded into the 65x65 S matrix).  u = S @ xa_q computed once.
  - f tiles [128 keys, 512 q] from PE (f32r, 1 cyc/row), exp split between
    ScalarE (exact, bf16 out) and VectorE (Schraudolph bit-trick exp producing
    bf16 via uint16 integer arithmetic) to balance the two engines.
  - y accumulated TRANSPOSED: yt[128 q, 33] += e_chunk^T @ g_chunk with e as
    the stationary operand, so the PE streams only 33 columns per chunk
    (bf16, 1 cyc/row).  Col 32 accumulates softmax denominators.
  - 4 query-subtile accumulation chains share one PSUM bank (has_written
    per-element semantics: single start on the bank's first matmul).
  - epilogue per PAIR of 128-query tiles: GPSIMD normalize_recip (y/den,
    bf16), one PE transpose [128,64]->[64,128], block-diagonal output conv
    (bf16) producing both tiles' outputs as [128,128], residual add
    (+BN folded), DMA out in pair layout (host unscrambles).
  - epilogue stages are software-pipelined into the NEXT query block's tile
    loop; prep (u/gt convs) rides the f-psum pool slots under block 0.

Host folds BN into the output conv, rotates x per-core so the query block is
always columns 0:3200 (softmax is invariant to key permutation), augments x
with a ones-row, and stitches the 8 [64,3200] results back into [4,64,80,80].
"""

import numpy as np
import ml_dtypes

import concourse.bass as bass
import concourse.tile as tile
from concourse import bacc
from concourse import mybir
from concourse.bass import ts
from concourse.bass_utils import run_bass_kernel_spmd

B, C, HH, WW = 4, 64, 80, 80
N = HH * WW            # 6400 key positions per sample
NQ = N // 2            # 3200 query rows per core
INTER = 32
NCORES = 8
CA = C + 1             # channels + ones row

MC = 128               # keys per PE chunk
NMC = N // MC          # 50 chunks
PACK = 2               # chunks per f-tile (2 PSUM banks, one exp call per tile)
NB = 512               # query block size

F32 = mybir.dt.float32
F32R = mybir.dt.float32r
BF16 = mybir.dt.bfloat16
U16 = mybir.dt.uint16
EXP = mybir.ActivationFunctionType.Exp
COPY = mybir.ActivationFunctionType.Copy
ADD = mybir.AluOpType.add
MULT = mybir.AluOpType.mult

BN_EPS = 1e-4

# Schraudolph fast-exp constants targeting the bf16 bit layout:
#   j = round(A16 * f + B16);  bitcast_u16_to_bf16(j) ~= exp(f)
A16 = float(2.0**7 / np.log(2.0))
C16 = 7.42
B16 = float(127 * 128 - C16)

# exp engine per f-tile (25 tiles of 2 chunks per block): 'A' = ScalarE
# exact exp, 'D' = DVE Schraudolph.  Whole-tile assignment so each engine
# releases its own PSUM slots at its own pace.  Tuned for busy balance.
TILE_ENG = list("DADAA" * 5)
# tile indices of the next block under which the previous block's epilogue
# q-tile stages are emitted: A = recip+scale (DVE), B = transpose/conv/resid
FLUSH_DELAY = 3
EPI_A = {3: 0, 11: 1}
EPI_B = {6: 0, 14: 1}
EPI_C = {9: 0, 17: 1}

ATT_BLOCKS = [(0, 512), (512, 512), (1024, 512), (1536, 512), (2048, 512),
              (2560, 384), (2944, 256)]

# epilogue processes query-tiles in PAIRS (block-diag output conv produces
# both tiles' outputs as one [128,128]): (block, first qtile, has second)
PAIRS = []
for _bi, (_n0, _nb) in enumerate(ATT_BLOCKS):
    _ntl = _nb // 128
    _q = 0
    while _q < _ntl:
        PAIRS.append((_bi, _q, _q + 1 < _ntl))
        _q += 2
PAIR_W = len(PAIRS) * 128
PAIR_BASE = {}
for _p, (_bi, _qa, _f) in enumerate(PAIRS):
    PAIR_BASE.setdefault(_bi, _p)

GROUPS = []
_c0 = 0
while _c0 < NMC:
    _g = min(PACK, NMC - _c0)
    GROUPS.append((_c0, _g))
    _c0 += _g


def _blocks(total, size):
    off = 0
    while off < total:
        sz = min(size, total - off)
        yield off, sz
        off += sz


def _emit(tc, d):
    nc = tc.nc

    with tc.tile_pool(name="singles", bufs=1) as singles:
        # small first chunk so the startup dependency chain clears quickly
        xa = singles.tile([CA, N], F32, tag="xa")
        XA_CHUNKS = [(0, 512), (512, 1088), (1600, 1600), (3200, 1600), (4800, 1600)]
        nc.gpsimd.dma_start(xa[:, 0:512], d["xa"][:, 0:512])
        s_t = singles.tile([CA, CA], F32, tag="s_t")
        nc.gpsimd.dma_start(s_t[:], d["s"][:])
        wga = singles.tile([CA, INTER + 1], F32, tag="wga")
        nc.gpsimd.dma_start(wga[:], d["wga"][:])
        for off, sz in XA_CHUNKS[1:]:
            nc.gpsimd.dma_start(xa[:, off : off + sz], d["xa"][:, off : off + sz])
        wol = singles.tile([INTER, C], BF16, tag="wol")
        nc.gpsimd.dma_start(wol[:], d["wol"][:])
        ident = singles.tile([128, 128], BF16, tag="ident")
        nc.gpsimd.dma_start(ident[:], d["ident"][:])
        xr = singles.tile([C, NQ], F32, tag="xr")
        for off, sz in _blocks(NQ, 1600):
            nc.gpsimd.dma_start(xr[:, off : off + sz], d["xr"][:, off : off + sz])

        # f32r operands must come from a rounding producer (DVE/ACT), not DMA.
        # Key-half pieces (cols 3200:6400) are emitted inside block 0's tile
        # loop — they're not needed until f-tile 12.
        xa_r = singles.tile([CA, N], F32R, tag="xa_r")
        nc.vector.tensor_copy(xa_r[:, 0:512], xa[:, 0:512])
        s_r = singles.tile([CA, CA], F32R, tag="s_r")
        nc.vector.tensor_copy(s_r[:], s_t[:])
        sr = s_r[:]
        nc.vector.tensor_copy(xa_r[:, 512:1600], xa[:, 512:1600])
        nc.vector.tensor_copy(xa_r[:, 1600:3200], xa[:, 1600:3200])
        nc.scalar.activation(xa_r[:, 3200:4800], xa[:, 3200:4800], COPY)
        nc.scalar.activation(xa_r[:, 4800:N], xa[:, 4800:N], COPY)
        xar = xa_r[:]

        u = singles.tile([CA, NQ], F32R, tag="u")
        ur = u[:]
        gt = singles.tile([128, NMC, INTER + 1], BF16, tag="gt")
        GTC = 8  # gt chunks per PSUM tile

        # ---- attention ----
        # PSUM banks: fps 2x3 + ytp 1 + epi 1 = 8.  yt is copied to SBUF
        # right after its accumulation completes (one cheap DVE op), so the
        # epilogue reads SBUF and the yt bank frees immediately.  Block b's
        # epilogue is emitted interleaved into block b+1's group loop so its
        # latency chain hides under the next block's compute.
        with tc.tile_pool(name="fps", bufs=3, space="PSUM") as fps, tc.tile_pool(
            name="ytp", bufs=1, space="PSUM"
        ) as ytp, tc.tile_pool(name="epi", bufs=1, space="PSUM") as epi, tc.tile_pool(
            name="esb", bufs=5
        ) as esb, tc.tile_pool(name="ep", bufs=2) as ep:
            ytsbs = {}
            ostage = {}
            mtiles = {}

            # prep work (u = S @ xa_q; gt = xa^T @ wga in bf16) rides the fps
            # pool's slots: pu(0)/pg(0) run before block 0, the rest are
            # injected between block 0's tiles so attention starts immediately
            def emit_pu(k):
                off = k * NB
                sz = min(NB, NQ - off)
                pu = fps.tile([CA, NB], F32, tag="f", name=f"pu_{k}")
                nc.tensor.matmul(
                    pu[:, :sz], lhsT=sr, rhs=xar[:, off : off + sz],
                    start=True, stop=True,
                )
                if k % 2 == 0:
                    nc.scalar.activation(u[:, off : off + sz], pu[:, :sz], COPY)
                else:
                    nc.vector.tensor_copy(u[:, off : off + sz], pu[:, :sz])

            def emit_pg(k):
                kg0 = k * GTC
                take = min(GTC, NMC - kg0)
                pg = fps.tile([128, GTC, INTER + 1], F32, tag="f", name=f"pg_{k}")
                for j in range(take):
                    nc.tensor.matmul(
                        pg[:, j, :], lhsT=xa[:, ts(kg0 + j, MC)], rhs=wga[:],
                        start=True, stop=True,
                    )
                if k % 2 == 0:
                    nc.vector.tensor_copy(gt[:, kg0 : kg0 + take, :], pg[:, :take, :])
                else:
                    nc.scalar.activation(gt[:, kg0 : kg0 + take, :], pg[:, :take, :], COPY)

            emit_pu(0)
            emit_pg(0)
            INJECT_PG = {2: 1, 6: 2, 10: 3, 14: 4, 18: 5, 22: 6}
            INJECT_PU = {1: 1, 7: 2, 11: 3, 15: 4, 19: 5, 23: 6}

            def emit_epi_a(bi, k):
                """Epilogue stage A (GPSIMD): m = y / den for a q-tile pair."""
                bi_, qa, full = PAIRS[PAIR_BASE[bi] + k]
                assert bi_ == bi
                ytsb = ytsbs[bi]
                m = ep.tile([128, 2 * INTER], BF16, tag="m", name=f"m_{bi}_{k}")
                nc.gpsimd.normalize_recip(
                    m[:, 0:INTER], ytsb[:, qa, 0:INTER], ytsb[:, qa, INTER : INTER + 1]
                )
                if full:
                    nc.gpsimd.normalize_recip(
                        m[:, INTER : 2 * INTER], ytsb[:, qa + 1, 0:INTER],
                        ytsb[:, qa + 1, INTER : INTER + 1],
                    )
                else:
                    nc.gpsimd.memset(m[:, INTER : 2 * INTER], 0.0)
                mtiles[(bi, k)] = m

            yntiles = {}

            def emit_epi_b(bi, k):
                """Epilogue stage B: transpose pair to [64,128], copy to SBUF."""
                m = mtiles.pop((bi, k))
                tr = epi.tile([2 * INTER, 128], BF16, tag="z", name=f"tr_{bi}_{k}")
                nc.tensor.transpose(tr[:], m[:], ident[:])
                yn = ep.tile([2 * INTER, 128], BF16, tag="yn", name=f"yn_{bi}_{k}")
                nc.vector.tensor_copy(yn[:], tr[:])
                yntiles[(bi, k)] = yn

            def emit_epi_c(bi, k):
                """Epilogue stage C: block-diag output conv, residual, dma."""
                p = PAIR_BASE[bi] + k
                yn = yntiles.pop((bi, k))
                z = epi.tile([128, 128], F32, tag="z", name=f"z_{bi}_{k}")
                nc.tensor.matmul(z[:], lhsT=wo2[:], rhs=yn[:], start=True, stop=True)
                o = ep.tile([128, 128], F32, tag="o", name=f"o_{bi}_{k}")
                nc.vector.tensor_tensor(
                    o[:], z[:], xr2[:, p * 128 : (p + 1) * 128], op=ADD
                )
                nc.sync.dma_start(d["out2"][:, p * 128 : (p + 1) * 128], o[:])

            for bi, (n0, nb) in enumerate(ATT_BLOCKS):
                ntl = nb // 128
                yt = ytp.tile([128, 4, INTER + 1], F32, tag="yt", name=f"yt_{bi}")
                prev_npr = (ATT_BLOCKS[bi - 1][1] // 128 + 1) // 2 if bi > 0 else 0
                pendq = []

                def flush_y(yt=yt, nb=nb, ntl=ntl, pendq=pendq):
                    e16, c0p, gszp = pendq.pop(0)
                    for j in range(gszp):
                        ch = c0p + j
                        for qi in range(ntl):
                            nc.tensor.matmul(
                                yt[:, qi, :],
                                lhsT=e16[:, j, qi * 128 : (qi + 1) * 128].bitcast(BF16),
                                rhs=gt[:, ch, :],
                                start=(ch == 0 and qi == 0),
                                stop=(ch == NMC - 1 and qi == ntl - 1),
                                skip_group_check=True,
                            )

                for gi, (c0g, gsz) in enumerate(GROUPS):
                    pf = fps.tile([128, PACK, NB], F32, tag="f", name=f"pf_{bi}_{gi}")
                    for j in range(gsz):
                        nc.tensor.matmul(
                            pf[:, j, :nb],
                            lhsT=xar[:, ts(c0g + j, MC)],
                            rhs=ur[:, n0 : n0 + nb],
                            start=True, stop=True,
                        )
                    # yt matmuls delayed by 2 tiles: their exp is done by the
                    # time the in-order PE SEQ reaches them, so no SEQ stall
                    if len(pendq) >= FLUSH_DELAY:
                        flush_y()
                    e16 = esb.tile([128, PACK, NB], U16, tag="e", name=f"e_{bi}_{gi}")
                    if TILE_ENG[gi] == "A":
                        nc.scalar.activation(
                            e16[:, 0:gsz, :nb].bitcast(BF16), pf[:, 0:gsz, :nb], EXP
                        )
                    else:
                        nc.vector.tensor_scalar(
                            e16[:, 0:gsz, :nb], pf[:, 0:gsz, :nb],
                            A16, B16, op0=MULT, op1=ADD,
                        )
                    pendq.append((e16, c0g, gsz))
                    # block 0: inject remaining prep under the early tiles
                    if bi == 0:
                        if gi in INJECT_PG:
                            emit_pg(INJECT_PG[gi])
                        if gi in INJECT_PU:
                            emit_pu(INJECT_PU[gi])

                    # interleave previous block's epilogue under this block
                    if bi > 0:
                        if gi in EPI_A and EPI_A[gi] < prev_npr:
                            emit_epi_a(bi - 1, EPI_A[gi])
                        if gi in EPI_B and EPI_B[gi] < prev_npr:
                            emit_epi_b(bi - 1, EPI_B[gi])
                        if gi in EPI_C and EPI_C[gi] < prev_npr:
                            emit_epi_c(bi - 1, EPI_C[gi])
                while pendq:
                    flush_y()
                ytsb = ep.tile([128, 4, INTER + 1], F32, tag="ytsb", name=f"ytsb_{bi}")
                nc.vector.tensor_copy(ytsb[:, :ntl, :], yt[:, :ntl, :])
                ytsbs[bi] = ytsb
            last = len(ATT_BLOCKS) - 1
            last_npr = (ATT_BLOCKS[-1][1] // 128 + 1) // 2
            for k in range(last_npr):
                emit_epi_a(last, k)
            for k in range(last_npr):
                emit_epi_b(last, k)
            for k in range(last_npr):
                emit_epi_c(last, k)


def build():
    nc = bacc.Bacc("TRN2", target_bir_lowering=False, debug=False)
    d = {}
    d["xa"] = nc.dram_tensor("xa", [CA, N], F32, kind="ExternalInput").ap()
    d["xr2"] = nc.dram_tensor("xr2", [128, PAIR_W], F32, kind="ExternalInput").ap()
    d["s"] = nc.dram_tensor("s", [CA, CA], F32, kind="ExternalInput").ap()
    d["wga"] = nc.dram_tensor("wga", [CA, INTER + 1], F32, kind="ExternalInput").ap()
    d["wo2"] = nc.dram_tensor("wo2", [C, 128], BF16, kind="ExternalInput").ap()
    d["ident"] = nc.dram_tensor("ident", [128, 128], BF16, kind="ExternalInput").ap()
    d["out2"] = nc.dram_tensor("out2", [128, PAIR_W], F32, kind="ExternalOutput").ap()
    with tile.TileContext(nc) as tc:
        _emit(tc, d)
    nc.compile()
    return nc


def make_in_maps(x, w_theta, b_theta, w_phi, b_phi, w_g, b_g,
                 w_out, b_out, bn_gamma, bn_beta, bn_mean, bn_var):
    bf = ml_dtypes.bfloat16
    x = np.ascontiguousarray(np.asarray(x, dtype=np.float32))
    w_theta = np.asarray(w_theta, np.float32)
    b_theta = np.asarray(b_theta, np.float32)
    w_phi = np.asarray(w_phi, np.float32)
    b_phi = np.asarray(b_phi, np.float32)
    w_g = np.asarray(w_g, np.float32)
    b_g = np.asarray(b_g, np.float32)
    w_out = np.asarray(w_out, np.float32)
    b_out = np.asarray(b_out, np.float32)
    bn_gamma = np.asarray(bn_gamma, np.float32)
    bn_beta = np.asarray(bn_beta, np.float32)
    bn_mean = np.asarray(bn_mean, np.float32)
    bn_var = np.asarray(bn_var, np.float32)

    inv = bn_gamma / np.sqrt(bn_var + BN_EPS)
    wo_folded = w_out * inv[:, None]                       # [64,32]
    bo_folded = (b_out - bn_mean) * inv + bn_beta          # [64]

    theta_aug = np.concatenate([w_theta, b_theta[:, None]], axis=1)  # [32,65]
    phi_aug = np.concatenate([w_phi, b_phi[:, None]], axis=1)        # [32,65]
    s_aug = np.ascontiguousarray(
        (theta_aug.astype(np.float64).T @ phi_aug.astype(np.float64)).astype(np.float32)
    )                                                       # [65,65]
    wg_aug = np.zeros((CA, INTER + 1), np.float32)
    wg_aug[:C, :INTER] = w_g.T
    wg_aug[C, :INTER] = b_g
    wg_aug[C, INTER] = 1.0                                  # denominator ones col
    wo_l = wo_folded.T.astype(bf)                           # [32,64] bf16
    wo2 = np.zeros((C, 128), bf)                            # block-diag pair conv
    wo2[0:INTER, 0:C] = wo_l
    wo2[INTER : 2 * INTER, C : 2 * C] = wo_l
    ident = np.eye(128, dtype=bf)

    xflat = x.reshape(B, C, N)
    in_maps = []
    for core in range(NCORES):
        b, h = divmod(core, 2)
        xrot = np.roll(xflat[b], -h * NQ, axis=1)
        xa = np.ascontiguousarray(
            np.concatenate([xrot, np.ones((1, N), np.float32)], axis=0)
        )
        xres = xrot[:, :NQ] + bo_folded[:, None]
        xr2 = np.zeros((128, PAIR_W), np.float32)
        for p, (bi, qa, full) in enumerate(PAIRS):
            q0 = ATT_BLOCKS[bi][0] + qa * 128
            xr2[0:C, p * 128 : (p + 1) * 128] = xres[:, q0 : q0 + 128]
            if full:
                xr2[C:128, p * 128 : (p + 1) * 128] = xres[:, q0 + 128 : q0 + 256]
        in_maps.append(
            {
                "xa": xa,
                "xr2": np.ascontiguousarray(xr2),
                "s": s_aug,
                "wga": wg_aug,
                "wo2": wo2,
                "ident": ident,
            }
        )
    return in_maps


def assemble_out(results):
    out = np.empty((B, C, N), np.float32)
    for core in range(NCORES):
        b, h = divmod(core, 2)
        o2 = results[core]["out2"]                          # [128, PAIR_W]
        full_o = np.empty((C, NQ), np.float32)
        for p, (bi, qa, full) in enumerate(PAIRS):
            q0 = ATT_BLOCKS[bi][0] + qa * 128
            full_o[:, q0 : q0 + 128] = o2[0:C, p * 128 : (p + 1) * 128]
            if full:
                full_o[:, q0 + 128 : q0 + 256] = o2[C:128, p * 128 : (p + 1) * 128]
        out[b][:, h * NQ : (h + 1) * NQ] = full_o
    return out.reshape(B, C, HH, WW)


_NC_CACHE = [None]


def kernel(**inputs):
    if _NC_CACHE[0] is None:
        _NC_CACHE[0] = build()
    nc = _NC_CACHE[0]
    in_maps = make_in_maps(**inputs)
    res = run_bass_kernel_spmd(nc, in_maps, core_ids=list(range(NCORES)))
    return assemble_out(res.results)
